# revision 14
# baseline (speedup 1.0000x reference)
"""Trainium2 Bass kernel for nn_CombinedModel_wGCN (GNN message passing).

Reference computation per event b (B=4096 events, N=128 particles):
  x = concat(feat, emb_table[pdg])          [128, 16]
  x = x @ W_in + b_in                       [128, 128]
  6x: x = relu(x @ W_h[l] + b_h[l]); x = adj @ x
  out[b] = (mask-weighted mean_i x) @ W_out + b_out

Strategy (pure data-parallel over 8 cores, 512 events each, groups of 4):
  - State kept transposed per event: Xh_e = x_e^T [d, i] (bf16). The dense
    layer is per-event matmul(lhsT=Xh_e, rhs=W_h[l]) producing [j, d'] —
    which feeds the aggregation matmul(lhsT=R_e, rhs=adjT_e) directly, so
    the whole layer chain needs NO transposes.
  - Accuracy: bf16 activations, weights split into bf16 hi+lo pairs
    accumulated in PSUM (x@W_hi + x@W_lo), adjacency in bf16. Emulated
    end-to-end error ~4.5e-3 vs f32 reference.
  - Bias b_h: reference uses zeros; if nonzero at runtime, a rank-1
    matmul (ones ⊗ b) accumulates bias into the dense PSUM before relu.
  - Masked-mean pooling folds into v = adj^T (mask/denom) (host), so the
    last aggregation is an N=1 matmul per event into a persistent PSUM
    bank; the final W_out projection runs once in f32r over all 512
    pooled columns.
  - Groups are emitted in a 3-wide software-pipelined wavefront so the
    PE always has independent (LDWEIGHTS, MATMUL) pairs in flight.
"""

import os
import numpy as np
import ml_dtypes

B, N = 4096, 128
NUM_FEAT, EMBED = 8, 8
UNITS = 128
HIDDEN = 6
NCORES = 8
BC = B // NCORES  # events per core
G = 4  # events per group (one PSUM bank of 512 f32 columns)
NG = BC // G
D0 = NUM_FEAT + EMBED + 1  # input features augmented with ones row (b_in)
WF = 3  # wavefront width (groups in flight)

_cache = {}


def _build_nc(ngroups, has_bias):
    import concourse.tile as tile
    from concourse import mybir, bacc

    f32 = mybir.dt.float32
    f32r = mybir.dt.float32r
    bf16 = mybir.dt.bfloat16
    Relu = mybir.ActivationFunctionType.Relu
    GW = G * 128

    nc = bacc.Bacc(
        trn_type="TRN2", target_bir_lowering=False, debug=False, num_devices=NCORES
    )
    d_adjt = nc.declare_dram_parameter("adjt", [NG, 128, GW], bf16, isOutput=False)
    d_x0t = nc.declare_dram_parameter("x0t", [NG, D0, GW], bf16, isOutput=False)
    d_vt = nc.declare_dram_parameter("vt", [128, BC], bf16, isOutput=False)
    d_whh = nc.declare_dram_parameter("whh", [HIDDEN, 128, 128], bf16, isOutput=False)
    d_whl = nc.declare_dram_parameter("whl", [HIDDEN, 128, 128], bf16, isOutput=False)
    d_winh = nc.declare_dram_parameter("winh", [D0, 128], bf16, isOutput=False)
    d_winl = nc.declare_dram_parameter("winl", [D0, 128], bf16, isOutput=False)
    d_bh = nc.declare_dram_parameter("bh", [HIDDEN, 128], f32, isOutput=False)
    d_wout = nc.declare_dram_parameter("wout", [2, 128, 1], bf16, isOutput=False)
    d_bout = nc.declare_dram_parameter("bout", [1, 1], f32, isOutput=False)
    d_out = nc.declare_dram_parameter("out", [1, BC], f32, isOutput=True)

    with tile.TileContext(nc) as tc:
        with (
            tc.tile_pool(name="const", bufs=1) as constp,
            tc.tile_pool(name="adj", bufs=2 * WF + 2) as adjp,
            tc.tile_pool(name="x0", bufs=WF + 2) as x0p,
            tc.tile_pool(name="work", bufs=2 * WF) as workp,
            tc.tile_pool(name="ps", bufs=WF, space="PSUM") as psp,
            tc.tile_pool(name="psfix", bufs=1, space="PSUM") as psfixp,
        ):
            # ---- constants ----
            whh, whl = [], []
            for l in range(HIDDEN):
                th = constp.tile([128, 128], bf16, tag=f"whh{l}")
                nc.sync.dma_start(th[:], d_whh[l])
                whh.append(th)
                tl = constp.tile([128, 128], bf16, tag=f"whl{l}")
                nc.sync.dma_start(tl[:], d_whl[l])
                whl.append(tl)
            winh = constp.tile([D0, 128], bf16, tag="winh")
            nc.sync.dma_start(winh[:], d_winh[:])
            winl = constp.tile([D0, 128], bf16, tag="winl")
            nc.sync.dma_start(winl[:], d_winl[:])
            wouth = constp.tile([128, 1], bf16, tag="wouth")
            nc.sync.dma_start(wouth[:], d_wout[0])
            woutl = constp.tile([128, 1], bf16, tag="woutl")
            nc.sync.dma_start(woutl[:], d_wout[1])
            boutt = constp.tile([1, 1], f32, tag="bout")
            nc.sync.dma_start(boutt[:], d_bout[:])
            vsb = constp.tile([128, BC], bf16, tag="vsb")
            nc.sync.dma_start(vsb[:], d_vt[:])
            brow = []
            if has_bias:
                ones_row = constp.tile([1, 128], bf16, tag="ones_row")
                nc.vector.memset(ones_row[:], 1.0)
                for l in range(HIDDEN):
                    bst = constp.tile([1, 128], f32, tag=f"bst{l}")
                    nc.sync.dma_start(bst[:], d_bh[l].rearrange("(o d) -> o d", o=1))
                    bb = constp.tile([1, GW], bf16, tag=f"brow{l}")
                    for e in range(G):
                        nc.vector.tensor_copy(bb[:, e * 128 : (e + 1) * 128], bst[:])
                    brow.append(bb)

            pooled = psfixp.tile([128, BC], f32, tag="pooled")

            def copy_into(i, dst, src):
                if i % 2 == 0:
                    nc.scalar.copy(dst, src)
                else:
                    nc.vector.tensor_copy(dst, src)

            def relu_into(i, dst, src):
                if i % 2 == 0:
                    nc.scalar.activation(dst, src, Relu)
                else:
                    nc.vector.tensor_scalar_max(dst, src, 0.0)

            # ---- wavefront over groups of G events ----
            xh = {}
            for gb in range(0, ngroups, WF):
                gs = range(gb, min(gb + WF, ngroups))
                for g in gs:
                    adjt = adjp.tile([128, GW], bf16, tag="adjt")
                    nc.sync.dma_start(adjt[:], d_adjt[g])
                    x0t = x0p.tile([D0, GW], bf16, tag="x0t")
                    nc.sync.dma_start(x0t[:], d_x0t[g])
                    pin = psp.tile([128, GW], f32, tag="dense")
                    nc.tensor.matmul(pin[:], winh[:], x0t[:], start=True, stop=False)
                    nc.tensor.matmul(pin[:], winl[:], x0t[:], start=False, stop=True)
                    t = workp.tile([128, GW], bf16, tag="xh")
                    copy_into(g, t[:], pin[:])
                    xh[g] = (t, adjt)

                for l in range(HIDDEN):
                    pd = {}
                    for g in gs:
                        t, adjt = xh[g]
                        p = psp.tile([128, GW], f32, tag="dense")
                        for e in range(G):
                            s = slice(e * 128, (e + 1) * 128)
                            nc.tensor.matmul(
                                p[:, s], t[:, s], whh[l][:], start=True, stop=False
                            )
                            nc.tensor.matmul(
                                p[:, s], t[:, s], whl[l][:],
                                start=False, stop=not has_bias,
                            )
                        if has_bias:
                            nc.tensor.matmul(
                                p[:], ones_row[:], brow[l][:], start=False, stop=True,
                                skip_group_check=True,
                            )
                        pd[g] = p
                    rr = {}
                    for g in gs:
                        r = workp.tile([128, GW], bf16, tag="r")
                        relu_into(g + l, r[:], pd[g][:])
                        rr[g] = r
                    if l < HIDDEN - 1:
                        pa = {}
                        for g in gs:
                            _, adjt = xh[g]
                            p = psp.tile([128, GW], f32, tag="agg")
                            for e in range(G):
                                s = slice(e * 128, (e + 1) * 128)
                                nc.tensor.matmul(
                                    p[:, s], rr[g][:, s], adjt[:, s],
                                    start=True, stop=True,
                                )
                            pa[g] = p
                        for g in gs:
                            t = workp.tile([128, GW], bf16, tag="xh")
                            copy_into(g + l + 1, t[:], pa[g][:])
                            xh[g] = (t, xh[g][1])
                    else:
                        for g in gs:
                            for e in range(G):
                                s = slice(e * 128, (e + 1) * 128)
                                ev = g * G + e
                                nc.tensor.matmul(
                                    pooled[:, ev : ev + 1],
                                    rr[g][:, s],
                                    vsb[:, ev : ev + 1],
                                    start=True, stop=True,
                                )
                xh.clear()

            # ---- final projection: out = pooled^T @ W_out + b_out ----
            # bf16 hi/lo split keeps the whole kernel fp32-free (FWL-friendly)
            psb = constp.tile([128, BC], f32, tag="psb")
            nc.vector.tensor_copy(psb[:], pooled[:])
            phi = constp.tile([128, BC], bf16, tag="phi")
            nc.scalar.copy(phi[:], psb[:])
            plo = constp.tile([128, BC], bf16, tag="plo")
            nc.vector.tensor_tensor(
                plo[:], psb[:], phi[:], mybir.AluOpType.subtract
            )
            pout = psfixp.tile([1, BC], f32, tag="pout")
            nc.tensor.matmul(pout[:], wouth[:], phi[:], start=True, stop=False)
            nc.tensor.matmul(pout[:], wouth[:], plo[:], start=False, stop=False)
            nc.tensor.matmul(pout[:], woutl[:], phi[:], start=False, stop=True)
            outsb = constp.tile([1, BC], f32, tag="outsb")
            nc.vector.tensor_scalar_add(outsb[:], pout[:], boutt[:])
            nc.sync.dma_start(d_out[:], outsb[:])

    nc.finalize()
    return nc


def _split2(w):
    bf = ml_dtypes.bfloat16
    hi = w.astype(bf)
    lo = (w - hi.astype(np.float32)).astype(bf)
    return hi, lo


def _prep_inputs(pdg, feat, adj, mask, emb_table, W_in, b_in, W_h, b_h, W_out, b_out):
    bf = ml_dtypes.bfloat16
    pdg = np.asarray(pdg)
    feat = np.asarray(feat, dtype=np.float32)
    adj = np.asarray(adj, dtype=np.float32)
    mask = np.asarray(mask, dtype=np.float32)
    emb_table = np.asarray(emb_table, dtype=np.float32)

    emb = emb_table[pdg]  # [B, N, EMBED]
    ones = np.ones((B, N, 1), dtype=np.float32)
    x0 = np.concatenate([feat, emb, ones], axis=-1)  # [B, N, 17]
    x0t = x0.transpose(0, 2, 1)  # [B, 17, N]
    x0t4 = (
        np.ascontiguousarray(x0t.reshape(B // G, G, D0, N).transpose(0, 2, 1, 3))
        .reshape(B // G, D0, G * N)
        .astype(bf)
    )

    adjt = adj.transpose(0, 2, 1).astype(bf)  # [B, j, i]
    adjt4 = np.ascontiguousarray(
        adjt.reshape(B // G, G, N, N).transpose(0, 2, 1, 3)
    ).reshape(B // G, N, G * N)

    denom = np.clip(mask.sum(axis=1, keepdims=True), 1.0, None)
    m_scaled = (mask / denom).astype(np.float32)  # [B, N]
    v = np.matmul(m_scaled[:, None, :], adj).squeeze(1)  # [B, N]
    vt = v.T.astype(bf)  # [N, B]

    win_aug = np.concatenate(
        [np.asarray(W_in, np.float32), np.asarray(b_in, np.float32)[None, :]], axis=0
    )  # [17, 128]
    winh, winl = _split2(win_aug)
    whh, whl = _split2(np.asarray(W_h, np.float32))
    wouth, woutl = _split2(np.asarray(W_out, np.float32).reshape(128, 1))
    wout2 = np.stack([wouth, woutl])  # [2, 128, 1] bf16

    in_maps = []
    for c in range(NCORES):
        ev = slice(c * BC, (c + 1) * BC)
        gv = slice(c * (BC // G), (c + 1) * (BC // G))
        in_maps.append(
            {
                "adjt": adjt4[gv],
                "x0t": x0t4[gv],
                "vt": np.ascontiguousarray(vt[:, ev]),
                "whh": whh,
                "whl": whl,
                "winh": winh,
                "winl": winl,
                "bh": np.asarray(b_h, np.float32),
                "wout": wout2,
                "bout": np.asarray(b_out, np.float32).reshape(1, 1),
            }
        )
    return in_maps


def kernel(pdg, feat, adj, mask, emb_table, W_in, b_in, W_h, b_h, W_out, b_out):
    from concourse.bass_utils import run_bass_kernel_spmd

    ngroups = int(os.environ.get("KERNEL_NGROUPS", NG))
    has_bias = bool(np.any(np.asarray(b_h)))
    key = ("nc", ngroups, has_bias)
    if key not in _cache:
        _cache[key] = _build_nc(ngroups, has_bias)
    nc = _cache[key]

    in_maps = _prep_inputs(
        pdg, feat, adj, mask, emb_table, W_in, b_in, W_h, b_h, W_out, b_out
    )
    trace = bool(int(os.environ.get("KERNEL_TRACE", "0")))
    if trace:
        try:
            tmpdir = os.environ.get("KERNEL_TRACE_DIR") or None
            res = run_bass_kernel_spmd(
                nc, in_maps, core_ids=list(range(NCORES)), trace=True, tmpdir=tmpdir
            )
            _cache["last_exec_time_ns"] = res.exec_time_ns
            _cache["last_results"] = res
        except Exception as e:
            print(f"trace run failed ({type(e).__name__}: {e}); rerunning untraced")
            _cache["last_exec_time_ns"] = None
            res = run_bass_kernel_spmd(nc, in_maps, core_ids=list(range(NCORES)))
    else:
        res = run_bass_kernel_spmd(nc, in_maps, core_ids=list(range(NCORES)))
    out = np.concatenate([res.results[c]["out"].reshape(BC) for c in range(NCORES)])
    return out.reshape(B, 1).astype(np.float32)
